# revision 68
# baseline (speedup 1.0000x reference)
"""Causal self-attention with RoPE on 8 Trainium2 NeuronCores.

Sharding: batch (2) x head-groups (4 of 4 heads) -> 8 cores. Each core
projects its batch-half of x against its 4 heads' slice of w_qkv, runs
causal flash-style attention for those heads, and applies its slice of
w_out, producing a partial [2048, 1024] output (bf16). Host sums the 4
partials per batch in f32.

All matmul operands are bf16 (1 cycle/row on PE at any tile size; psum
accumulation stays f32), which also unlocks the DVE 2x/4x fast modes for
the SBUF-resident elementwise RoPE/mask work and halves HBM traffic.

On-device layouts (per core, S=2048, 4 heads, hd=64):
  x       [128, 8*512] bf16 per t-chunk (dim on partitions), one merged
          DMA per chunk (per-quarter for chunk 0 so PE starts ~4us in)
  QT/KT   2 x 4 x [128, 512] bf16, rows = 2 heads x 64, RoPE'd. The
          rotate-half matmul reuses its own projection psum bank (WAR
          implied by the raw copy) and is emitted one unit late so it
          never head-of-line blocks PE.
  Vsb     4 x [128, 4*128] bf16 in [k, d] layout, key-blocks of
          [1*64|v_h]*4: 64 ones cols per head -> AV psum rows 0:64 all
          hold the softmax denominator (no partition broadcast needed;
          matmul cost only depends on the moving free size, so the wide
          ones block is free). V is projected directly in [token, d]
          orientation (lhsT=x-block, rhs=wv^T) - no PE transposes; one
          strided copy per token-block drops it into place.
  scores  S^T [k, q] via matmul(lhsT=KT slice, rhs=QT slice); both heads
          of a pair write one shared [128,1024] psum tile so a single
          ACT exp covers both (diag blocks via one strided 3D act);
          causal diagonal squares masked post-exp by one strided bf16
          tri multiply on DVE. AVs run three blocks behind their scores
          (lag-3) to hide exp latency.
  ctx^T   accumulated [128, 512] psum per (head, q-chunk); rows 0:64 =
          denominator, 64:128 = context; normalized by DVE
          fast-reciprocal straight off psum (inputs kept at partition
          base 0 - custom DVE ops break on offset bases) + one multiply
  out     ctxT [256, 2048] x w_outT -> partial [2048, 1024] bf16; the
          last q-chunk runs a block-wise finish interleaved with its
          out-proj units so the kernel tail drains incrementally, with
          psum-pool alternation and DVE/Act split copies.

Chunk 0 projects c8-major (one unit per 128-dim contraction slice,
accumulating all 4 qk / 4 v psum groups at once, borrowing the idle
score/ctx psum banks) so PE starts as soon as the first wqk/x quarter
lands. NB: a matmul's start=True zeroes its whole 2KB psum BANK, so
concurrent accumulation groups must live in separate banks.

Projection, attention, and output projection are emitted as interleaved
unit streams (attention(qc) with projection(qc+1), out-proj(qc) with
attention(qc+1)) so PE stays dense. Within an attention unit the AVs
precede the scores (AV-B first: its exp wait subsumes AV-A's).
"""

import numpy as np

DIM = 1024
NUM_HEADS = 16
HEAD_DIM = 64
ROPE_BASE = 10000.0
B = 2
S = 2048
N_CORES = 8
HPC = 4            # heads per core
GROUPS = 2         # batch groups
CPG = N_CORES // GROUPS  # cores per group
TC = 512           # t-chunk (tokens per projection chunk)
NTC = S // TC      # 4
KB = 128           # key block
NKB = S // KB      # 16
VW = 128           # v block width per head (64 v cols + 64 ones cols)
VBW = HPC * VW     # 512

_compiled = None


def _build_nc(debug_dump=False):
    import concourse.tile as tile
    from concourse import bacc, mybir
    from contextlib import ExitStack

    f32 = mybir.dt.float32
    bf16 = mybir.dt.bfloat16
    AF = mybir.ActivationFunctionType

    nc = bacc.Bacc("TRN2", target_bir_lowering=False, debug=False,
                   num_devices=N_CORES)
    if debug_dump:
        dbg_qt0 = nc.dram_tensor("dbg_qt0", [128, S], f32, kind="ExternalOutput").ap()
        dbg_kt0 = nc.dram_tensor("dbg_kt0", [128, S], f32, kind="ExternalOutput").ap()
        dbg_vsb = nc.dram_tensor("dbg_vsb", [128, NTC * 4 * VBW], f32, kind="ExternalOutput").ap()
        dbg_ctx0 = nc.dram_tensor("dbg_ctx0", [128, S], f32, kind="ExternalOutput").ap()

    xT = nc.dram_tensor("xT", [DIM, S], bf16, kind="ExternalInput").ap()
    wqkT = nc.dram_tensor("wqkT", [128, 8 * 512], bf16, kind="ExternalInput").ap()
    wvT = nc.dram_tensor("wvT", [128, 8 * 256], bf16, kind="ExternalInput").ap()
    woT = nc.dram_tensor("woT", [128, 2 * 1024], bf16, kind="ExternalInput").ap()
    csT = nc.dram_tensor("csT", [128, 2 * S], bf16, kind="ExternalInput").ap()
    perm = nc.dram_tensor("perm", [128, 128], bf16, kind="ExternalInput").ap()
    tri = nc.dram_tensor("tri", [128, 128], bf16, kind="ExternalInput").ap()
    out_p = nc.dram_tensor("out_p", [S, DIM], bf16, kind="ExternalOutput").ap()

    with tile.TileContext(nc) as tc:
        with ExitStack() as ctx:
            big = ctx.enter_context(tc.tile_pool(name="big", bufs=1))

            wqk_sb = big.tile([128, 8 * 512], bf16, tag="wqk")
            wv_sb = big.tile([128, 8 * 256], bf16, tag="wv")
            wo_sb = big.tile([128, 2 * 1024], bf16, tag="wo")
            cs_sb = big.tile([128, 2 * S], bf16, tag="cs")  # [cos | sin]
            perm_sb = big.tile([128, 128], bf16, tag="perm")
            tri_sb = big.tile([128, 128], bf16, tag="tri")

            # per-chunk persistent tiles so phase 2 can start while phase 1
            # is still projecting later chunks
            qt = [[big.tile([128, TC], bf16, tag=f"qt{i}_{j}", name=f"qt{i}_{j}")
                   for j in range(NTC)] for i in range(2)]
            kt = [[big.tile([128, TC], bf16, tag=f"kt{i}_{j}", name=f"kt{i}_{j}")
                   for j in range(NTC)] for i in range(2)]
            vsb = [big.tile([128, 4 * VBW], bf16, tag=f"vsb{j}", name=f"vsb{j}")
                   for j in range(NTC)]
            ctxc = [[big.tile([128, TC], bf16, tag=f"ctx{i}_{j}", name=f"ctxc{i}_{j}")
                     for j in range(NTC)] for i in range(2)]

            with (
                tc.tile_pool(name="p1x", bufs=3) as p1x,
                tc.tile_pool(name="p1sb", bufs=3) as p1sb,
                tc.tile_pool(name="p2sb", bufs=6) as p2sb,
                tc.tile_pool(name="pps", bufs=2, space="PSUM") as pps,
                tc.tile_pool(name="sps", bufs=2, space="PSUM") as sps,
                tc.tile_pool(name="ctxps", bufs=2, space="PSUM") as ctxps,
            ):
                xtiles = [None] * NTC

                def x_dma(tci, xt_t, clo, chi):
                    """One merged DMA for c8-blocks [clo, chi) of chunk tci.
                    dst [p, c*TC + t] <- xT[(c*128 + p), tci*TC + t]."""
                    dst = xt_t[:, clo * TC:chi * TC].rearrange(
                        "p (c t) -> p c t", c=chi - clo)
                    src = xT[clo * 128:chi * 128,
                             tci * TC:(tci + 1) * TC].rearrange(
                        "(c p) t -> p c t", p=128)
                    nc.sync.dma_start(dst, src)

                def cs_dma(tci):
                    """cos+sin chunk slices in one 2-segment DMA."""
                    w = slice(tci * TC, (tci + 1) * TC)
                    dst = cs_sb[:].rearrange("p (s t) -> p s t", s=2)[:, :, w]
                    srcv = csT[:].rearrange("p (s t) -> p s t", s=2)[:, :, w]
                    nc.sync.dma_start(dst, srcv)

                def p1_dma(tci):
                    """Issue the x-chunk + cos/sin DMAs for chunk tci."""
                    xt_t = p1x.tile([128, 8 * TC], bf16, tag="x",
                                    name=f"x_{tci}")
                    x_dma(tci, xt_t, 0, 8)
                    if tci > 0:
                        cs_dma(tci)
                    xtiles[tci] = xt_t
                    return xt_t

                # ---- priority-ordered preamble ----
                # the first qk matmuls consume wqk block c8 + x0 block c8 in
                # order; stream them as interleaved 2-block quarters so PE
                # starts ~3us in, everything else in consumption order behind.
                xt0 = p1x.tile([128, 8 * TC], bf16, tag="x", name="x_0")
                # first c8-block as its own small pieces so the very first
                # matmul's deps land ~1.3us earlier than quarter granularity
                nc.sync.dma_start(wqk_sb[:, 0:512], wqkT[:, 0:512])
                x_dma(0, xt0, 0, 1)
                nc.sync.dma_start(wqk_sb[:, 512:1024], wqkT[:, 512:1024])
                x_dma(0, xt0, 1, 2)
                for q4 in range(1, 4):
                    nc.sync.dma_start(
                        wqk_sb[:, q4 * 1024:(q4 + 1) * 1024],
                        wqkT[:, q4 * 1024:(q4 + 1) * 1024])
                    x_dma(0, xt0, 2 * q4, 2 * q4 + 2)
                xtiles[0] = xt0
                nc.sync.dma_start(wv_sb[:], wvT[:])
                nc.sync.dma_start(perm_sb[:], perm[:])
                cs_dma(0)
                nc.sync.dma_start(tri_sb[:], tri[:])
                p1_dma(1)

                # ones-column blocks interleaved in v tiles: 64 ones cols per
                # head block (cols 0:64, before the 64 v cols) -> AV psum rows
                # 0:64 all hold the denominator, so normalization needs no
                # partition broadcast and the reciprocal's input stays at
                # partition base 0 (custom DVE ops break on offset bases).
                for j in range(NTC):
                    ones_view = vsb[j][:].rearrange(
                        "p (g w) -> p g w", w=VW)[:, :, 0:64]
                    nc.gpsimd.memset(ones_view, 1.0)

                scale = float(HEAD_DIM) ** -0.5

                rope_pend = []

                def rope_flush():
                    """RoPE tail of the previous qk unit: emitted one unit
                    later so the rot matmul never head-of-line blocks PE
                    behind its raw copy. rot reuses the unit's own qk psum
                    region (its WAR is implied by the raw-copy data dep), so
                    the pps pool never runs dry."""
                    if not rope_pend:
                        return
                    raw, rot_ps, tci, blk = rope_pend.pop(0)
                    cosw = cs_sb[:, tci * TC:(tci + 1) * TC]
                    sinw = cs_sb[:, S + tci * TC:S + (tci + 1) * TC]
                    nc.tensor.matmul(rot_ps[:], perm_sb[:], raw[:], start=True, stop=True)
                    t1 = p1sb.tile([128, TC], bf16, tag="t1", name=f"t1_{tci}_{blk}")
                    nc.gpsimd.tensor_mul(t1[:], raw[:], cosw)  # SBUF-only -> Pool
                    t2 = p1sb.tile([128, TC], bf16, tag="t2", name=f"t2_{tci}_{blk}")
                    nc.vector.tensor_mul(t2[:], rot_ps[:], sinw)
                    dest = (qt if blk < 2 else kt)[blk % 2][tci]
                    nc.vector.tensor_add(dest[:], t1[:], t2[:])

                def p1_qk_unit(xts, tci, blk):
                    qk_ps = pps.tile([128, TC], f32, tag="p", name=f"qk{tci}_{blk}")
                    for c8 in range(8):
                        nc.tensor.matmul(
                            qk_ps[:],
                            wqk_sb[:, c8 * 512 + blk * 128:c8 * 512 + blk * 128 + 128],
                            xts[:, c8 * TC:(c8 + 1) * TC],
                            start=(c8 == 0), stop=(c8 == 7))
                    raw = p1sb.tile([128, TC], bf16, tag="raw", name=f"raw{tci}_{blk}", bufs=4)
                    nc.vector.tensor_copy(raw[:], qk_ps[:])
                    rope_pend.append((raw, qk_ps, tci, blk))

                def p1_v_unit(xts, tci, tb):
                    """V for token-block tb, projected directly as [tok, d]."""
                    v_ps = pps.tile([128, 256], f32, tag="p", name=f"vp{tci}_{tb}")
                    for c8 in range(8):
                        nc.tensor.matmul(
                            v_ps[:],
                            xts[:, c8 * TC + tb * 128:c8 * TC + (tb + 1) * 128],
                            wv_sb[:, c8 * 256:(c8 + 1) * 256],
                            start=(c8 == 0), stop=(c8 == 7))
                    dst = vsb[tci][:, tb * VBW:(tb + 1) * VBW].rearrange(
                        "p (h w) -> p h w", h=HPC)[:, :, 64:128]
                    src = v_ps[:].rearrange("p (h w) -> p h w", h=HPC)
                    nc.vector.tensor_copy(dst, src)

                def p1_units(tci):
                    xts = xtiles[tci] if xtiles[tci] is not None else p1_dma(tci)
                    units = []
                    for blk in range(4):
                        def qk_u(b=blk):
                            if b > 0:
                                rope_flush()
                            p1_qk_unit(xts, tci, b)
                        units.append(qk_u)
                    units.append(lambda: (rope_flush(), p1_v_unit(xts, tci, 0))[1])
                    for tb in range(1, 4):
                        units.append(lambda t=tb: p1_v_unit(xts, tci, t))
                    return units

                def p1_units_chunk0():
                    """Chunk 0 runs c8-major (one unit per 128-dim slice of
                    the contraction, hitting all 4 qk / 4 v psum groups) so
                    PE starts as soon as the first wqk/x0 quarter lands
                    instead of waiting for the whole 2MB. The 4 concurrent qk
                    groups borrow the score pool's banks, v groups the ctx
                    pool's — both idle until attention starts."""
                    xts = xtiles[0]
                    # each accumulation group needs its own psum BANK (the
                    # matmul start bit zeroes the whole bank): qk halves of a
                    # [128,1024] tile are one bank each; the four 256-wide v
                    # groups get four separate tiles (ctx pool + the idle pps)
                    qk_t = [sps.tile([128, 1024], f32, tag="s", name=f"qk0h{i}")
                            for i in range(2)]
                    v_t = ([ctxps.tile([128, 256], f32, tag="ctx", name=f"vp0h{i}")
                            for i in range(2)] +
                           [pps.tile([128, 256], f32, tag="p", name=f"vp0h{i + 2}")
                            for i in range(2)])

                    def qk_c8(c8):
                        for blk in range(4):
                            nc.tensor.matmul(
                                qk_t[blk // 2][:, (blk % 2) * 512:(blk % 2) * 512 + 512],
                                wqk_sb[:, c8 * 512 + blk * 128:c8 * 512 + blk * 128 + 128],
                                xts[:, c8 * TC:(c8 + 1) * TC],
                                start=(c8 == 0), stop=(c8 == 7))

                    def v_tb(tb):
                        for c8 in range(8):
                            nc.tensor.matmul(
                                v_t[tb][:],
                                xts[:, c8 * TC + tb * 128:c8 * TC + (tb + 1) * 128],
                                wv_sb[:, c8 * 256:(c8 + 1) * 256],
                                start=(c8 == 0), stop=(c8 == 7))

                    def qk_post(blk):
                        qk_ps = qk_t[blk // 2][:, (blk % 2) * 512:(blk % 2) * 512 + 512]
                        raw = p1sb.tile([128, TC], bf16, tag="raw", name=f"raw0_{blk}", bufs=4)
                        nc.vector.tensor_copy(raw[:], qk_ps)
                        rope_pend.append((raw, qk_ps, 0, blk))

                    def v_post(tb):
                        # Act copy: DVE is saturated with rope chains at the
                        # chunk-0/att(0) boundary, Act still nearly idle
                        dst = vsb[0][:, tb * VBW:(tb + 1) * VBW].rearrange(
                            "p (h w) -> p h w", h=HPC)[:, :, 64:128]
                        nc.scalar.activation(
                            dst, v_t[tb][:].rearrange("p (h w) -> p h w", h=HPC),
                            AF.Copy, scale=1.0)

                    # rope order q0,k0,q1,k1 (blk 0,2,1,3) so att(0) pair 0
                    # unblocks after two flushes; v runs tb-major so each
                    # token-block's vsb copy can start as soon as its own
                    # group completes (att(0) AVs need vsb[0] early)
                    post_order = [0, 2, 1, 3]
                    units = [lambda c=c8: qk_c8(c) for c8 in range(8)]
                    units.append(lambda: qk_post(post_order[0]))
                    for i in range(4):
                        def u(i=i):
                            if i < 3:
                                qk_post(post_order[i + 1])
                            rope_flush()
                            if i > 0:
                                v_post(i - 1)
                            v_tb(i)
                        units.append(u)
                    units.append(lambda: v_post(3))
                    return units

                def att_scores(streams, kb):
                    """Both streams' score matmuls for k-block kb into ONE
                    shared psum tile (A cols 0:n, B cols 512:512+n); one exp
                    covers both."""
                    st0 = streams[0]
                    qc, d0 = st0["qc"], st0["qc"] * 4
                    n0 = max(0, 128 * (kb - d0))
                    n = 512 - n0
                    s_ps = sps.tile([128, 1024], f32, tag="s",
                                    name=f"s{st0['h']}_{qc}_{kb}")
                    e_sb = p2sb.tile([128, 1024], bf16, tag="e",
                                     name=f"e{st0['h']}_{qc}_{kb}", bufs=5)
                    for si_, st in enumerate(streams):
                        ti, po = st["ti"], st["po"]
                        off = 512 * si_
                        nc.tensor.matmul(
                            s_ps[:, off:off + n],
                            kt[ti][kb // 4][po:po + 64, (kb % 4) * 128:(kb % 4) * 128 + 128],
                            qt[ti][qc][po:po + 64, n0:512],
                            start=True, stop=True)
                    if n == 512:
                        nc.scalar.activation(e_sb[:, 0:1024], s_ps[:, 0:1024],
                                             AF.Exp, scale=scale)
                    else:
                        ev = e_sb[:].rearrange("p (s n) -> p s n", s=2)[:, :, 0:n]
                        sv = s_ps[:].rearrange("p (s n) -> p s n", s=2)[:, :, 0:n]
                        nc.scalar.activation(ev, sv, AF.Exp, scale=scale)
                    if kb >= d0:
                        dv = e_sb[:].rearrange("p (s n) -> p s n", s=2)[:, :, 0:128]
                        tv = tri_sb[:].unsqueeze(1).broadcast_to([128, 2, 128])
                        nc.vector.tensor_mul(dv, dv, tv)
                    for si_, st in enumerate(streams):
                        st["pends"].append((kb, 512 * si_, n0, n, e_sb))

                def att_av(st):
                    h, qc = st["h"], st["qc"]
                    nkb = qc * 4 + 4
                    kb, o, n0, n, e_sb = st["pends"].pop(0)
                    nc.tensor.matmul(
                        st["ctx"][0:128, n0:512],
                        vsb[kb // 4][:, (kb % 4) * VBW + VW * h:(kb % 4) * VBW + VW * h + VW],
                        e_sb[:, o:o + n],
                        start=(kb == 0), stop=(kb == nkb - 1))

                def att_finish(st):
                    h, qc, ti, po = st["h"], st["qc"], st["ti"], st["po"]
                    rec = p2sb.tile([64, 512], f32, tag="rec", name=f"rec{h}_{qc}", bufs=2)
                    nc.vector.reciprocal_approx_fast(rec[:], st["ctx"][0:64, :])
                    nc.vector.tensor_mul(
                        ctxc[ti][qc][po:po + 64, :], st["ctx"][64:128, :], rec[:])

                def att_units(qc):
                    """Units for all 4 heads at q-chunk qc, two streams each.
                    AVs run two blocks behind their scores (lag-2) so the
                    exp of block kb has the scores+AVs of kb+1 to hide
                    behind; within a block stream B's AV goes first (its
                    exp wait subsumes stream A's)."""
                    units = []
                    for hp in range(2):
                        streams = [{
                            "h": h, "qc": qc, "ti": h // 2, "po": 64 * (h % 2),
                        } for h in (2 * hp, 2 * hp + 1)]

                        def mk_start(strs=streams, q=qc):
                            def u():
                                for st in strs:
                                    st["ctx"] = ctxps.tile(
                                        [128, 512], f32, tag="ctx",
                                        name=f"cps{st['h']}_{q}")
                                    st["pends"] = []
                            return u
                        units.append(mk_start())

                        nkb = qc * 4 + 4
                        for kb in range(nkb):
                            def mk_unit(strs=streams, k=kb):
                                def u():
                                    if len(strs[0]["pends"]) >= 3:
                                        att_av(strs[1])  # B first: its wait
                                        att_av(strs[0])  # subsumes A's
                                    att_scores(strs, k)
                                return u
                            units.append(mk_unit())

                        def mk_flush(strs=streams):
                            def u():
                                while strs[0]["pends"]:
                                    att_av(strs[1])
                                    att_av(strs[0])
                            return u

                        def mk_finish(strs=streams):
                            def u():
                                for st in strs:
                                    att_finish(st)
                            return u
                        # separate units so the interleaver can slot PE work
                        # between the AV drain and the DVE finish chain
                        units.append(mk_flush())
                        if qc == NTC - 1 and hp == 1:
                            # final pair: block-wise finish interleaved with
                            # the last out-proj units so the kernel tail
                            # drains incrementally
                            for b in range(4):
                                def fb(strs=streams, b=b):
                                    for st in strs:
                                        att_finish_block(st, b)
                                units.append(fb)
                                units.append(lambda t=12 + b:
                                             out_unit(t, 0, split_act=True))
                                units.append(lambda t=12 + b:
                                             out_unit(t, 1, split_act=True))
                        else:
                            units.append(mk_finish())
                    return units

                osb_state = {}

                def out_unit(tt, ec, split_act=False):
                    qc = tt // 4
                    if split_act and (tt + ec) % 2 == 1:
                        # tail: alternate psum pools (scores pool is idle by
                        # now) to double the effective o_ps buffering
                        o_ps = sps.tile([128, 512], f32, tag="s", name=f"o{tt}_{ec}")
                    else:
                        o_ps = pps.tile([128, 512], f32, tag="p", name=f"o{tt}_{ec}")
                    for dc in range(2):
                        nc.tensor.matmul(
                            o_ps[:],
                            ctxc[dc][qc][:, (tt % 4) * 128:(tt % 4) * 128 + 128],
                            wo_sb[:, dc * 1024 + ec * 512:dc * 1024 + ec * 512 + 512],
                            start=(dc == 0), stop=(dc == 1))
                    if split_act:
                        # [128,1024] staging + one DMA per token-tile (fewer
                        # HWDGE gens in the drain) with DVE/Act half-copies
                        if ec == 0:
                            osb_state[tt] = p2sb.tile([128, 1024], bf16, tag="osb2",
                                                      name=f"ob{tt}", bufs=4)
                        o_sb = osb_state[tt]
                        # whole copy on Act: DVE is saturated by the
                        # block-wise finish chain in the drain
                        nc.scalar.activation(
                            o_sb[:, ec * 512:(ec + 1) * 512], o_ps[:],
                            AF.Copy, scale=1.0)
                        if ec == 1:
                            nc.sync.dma_start(out_p[tt * 128:(tt + 1) * 128, :], o_sb[:])
                    else:
                        o_sb = p2sb.tile([128, 512], bf16, tag="osb",
                                         name=f"ob{tt}_{ec}", bufs=6)
                        nc.vector.tensor_copy(o_sb[:], o_ps[:])
                        nc.sync.dma_start(
                            out_p[tt * 128:(tt + 1) * 128, ec * 512:(ec + 1) * 512],
                            o_sb[:])

                def out_units(qc, split_act=False):
                    return [lambda t=tt, e=ec: out_unit(t, e, split_act=split_act)
                            for tt in range(4 * qc, 4 * qc + 4) for ec in range(2)]

                def att_finish_block(st, b):
                    """128-col slice of the finish: the matching out-proj
                    token-tile only needs this block of ctxc, so the final
                    out units unblock incrementally instead of waiting the
                    full 512-col reciprocal+mul chain."""
                    h, ti, po = st["h"], st["ti"], st["po"]
                    cols = slice(b * 128, (b + 1) * 128)
                    rec = p2sb.tile([64, 128], f32, tag="recb",
                                    name=f"recb{h}_{b}", bufs=4)
                    nc.vector.reciprocal_approx_fast(rec[:], st["ctx"][0:64, cols])
                    nc.vector.tensor_mul(
                        ctxc[ti][NTC - 1][po:po + 64, cols],
                        st["ctx"][64:128, cols], rec[:])

                def run_interleaved(a_units, b_units):
                    na, nb = len(a_units), len(b_units)
                    ia = ib = 0
                    while ia < na or ib < nb:
                        if ib >= nb or (ia < na and ia * nb <= ib * na):
                            a_units[ia](); ia += 1
                        else:
                            b_units[ib](); ib += 1

                def p1_units_with_dma(tci, wo_dma=False):
                    units = p1_units_chunk0() if tci == 0 else p1_units(tci)
                    if tci + 2 < NTC:
                        units.insert(1, lambda t=tci + 2: (p1_dma(t), None)[1])
                    if wo_dma:
                        units.insert(2, lambda: nc.sync.dma_start(wo_sb[:], woT[:]))
                    return units

                run_interleaved(p1_units_with_dma(0), [])
                run_interleaved(p1_units_with_dma(1, wo_dma=True), att_units(0))
                run_interleaved(p1_units_with_dma(2), att_units(1) + out_units(0))
                run_interleaved(p1_units_with_dma(3), att_units(2) + out_units(1))
                # out(3) units are emitted inside att_units(3)'s final-pair
                # tail (block-wise finish interleave)
                run_interleaved(att_units(3), out_units(2))

                if debug_dump:
                    dbsb = big.tile([128, S], f32, tag="dbsb")
                    def dump(dst, srcs):
                        off = 0
                        for src in srcs:
                            w = src.shape[-1]
                            nc.vector.tensor_copy(dbsb[:, off:off + w], src)
                            nc.sync.dma_start(dst[:, off:off + w], dbsb[:, off:off + w])
                            off += w
                    dump(dbg_qt0, [qt[0][j][:] for j in range(NTC)])
                    dump(dbg_kt0, [kt[0][j][:] for j in range(NTC)])
                    dump(dbg_ctx0, [ctxc[0][j][:] for j in range(NTC)])
                    for j in range(NTC):
                        nc.vector.tensor_copy(dbsb[:], vsb[j][:])
                        nc.sync.dma_start(
                            dbg_vsb[:, j * 4 * VBW:(j + 1) * 4 * VBW], dbsb[:, 0:4 * VBW])

    nc.compile()
    return nc


def _rope_tables():
    inv_freq = 1.0 / (ROPE_BASE ** (np.arange(0, HEAD_DIM, 2, dtype=np.float32) / HEAD_DIM))
    t = np.arange(S, dtype=np.float32)
    freqs = np.outer(t, inv_freq)                      # (S, 32)
    emb = np.concatenate([freqs, freqs], axis=-1)      # (S, 64)
    cos = np.cos(emb).astype(np.float32).T             # (64, S)
    sin = np.sin(emb).astype(np.float32).T
    return np.tile(cos, (2, 1)), np.tile(sin, (2, 1))  # (128, S)


def _perm_mat():
    p = np.zeros((128, 128), dtype=np.float32)
    for base in (0, 64):
        for d in range(32):
            p[base + d + 32, base + d] = -1.0          # rot[d] = -q[d+32]
        for d in range(32, 64):
            p[base + d - 32, base + d] = 1.0           # rot[d] = q[d-32]
    return p


def _bf16(a):
    import ml_dtypes
    return np.asarray(a, dtype=ml_dtypes.bfloat16)


def core_inputs(c, x, w_qkv, w_out, cos2, sin2, perm_np, tri_np):
    g = c // CPG
    hs = [HPC * (c % CPG) + i for i in range(HPC)]
    xTg = np.ascontiguousarray(x[g].T)                                   # (1024, 2048)

    qrows = np.concatenate([w_qkv[h * 64:(h + 1) * 64] for h in hs])     # (256, 1024)
    krows = np.concatenate([w_qkv[DIM + h * 64:DIM + (h + 1) * 64] for h in hs])
    vrows = np.concatenate([w_qkv[2 * DIM + h * 64:2 * DIM + (h + 1) * 64] for h in hs])
    wqk = np.concatenate([qrows, krows])                                 # (512, 1024)
    wqkT = np.ascontiguousarray(
        wqk.reshape(512, 8, 128).transpose(2, 1, 0).reshape(128, 8 * 512))
    wvT = np.ascontiguousarray(
        vrows.reshape(256, 8, 128).transpose(2, 1, 0).reshape(128, 8 * 256))

    didx = np.concatenate([np.arange(h * 64, (h + 1) * 64) for h in hs])  # (256,)
    woTh = w_out[:, didx].T                                               # (256, 1024)
    woT = np.ascontiguousarray(
        woTh.reshape(2, 128, DIM).transpose(1, 0, 2).reshape(128, 2 * DIM))

    return {
        "xT": _bf16(xTg), "wqkT": _bf16(wqkT), "wvT": _bf16(wvT),
        "woT": _bf16(woT), "csT": _bf16(np.concatenate([cos2, sin2], axis=1)),
        "perm": _bf16(perm_np), "tri": _bf16(tri_np),
    }


def make_in_maps(x, w_qkv, w_out):
    x = np.asarray(x, dtype=np.float32)
    w_qkv = np.asarray(w_qkv, dtype=np.float32)
    w_out = np.asarray(w_out, dtype=np.float32)
    cos2, sin2 = _rope_tables()
    perm_np = _perm_mat()
    k_idx = np.arange(128)[:, None]
    q_idx = np.arange(128)[None, :]
    tri_np = (q_idx >= k_idx).astype(np.float32)
    return [core_inputs(c, x, w_qkv, w_out, cos2, sin2, perm_np, tri_np)
            for c in range(N_CORES)]


def get_compiled():
    global _compiled
    if _compiled is None:
        _compiled = _build_nc()
    return _compiled


def gather(results):
    out = np.empty((B, S, DIM), dtype=np.float32)
    for g in range(GROUPS):
        acc = results[g * CPG]["out_p"].astype(np.float32)
        for c in range(g * CPG + 1, (g + 1) * CPG):
            acc += results[c]["out_p"].astype(np.float32)
        out[g] = acc
    return out


def kernel(x, w_qkv, w_out):
    from concourse.bass_utils import run_bass_kernel_spmd
    nc = get_compiled()
    in_maps = make_in_maps(x, w_qkv, w_out)
    res = run_bass_kernel_spmd(nc, in_maps, list(range(N_CORES)))
    return gather(res.results)
